# revision 24
# baseline (speedup 1.0000x reference)
"""Self-contained TRN2 Bass kernel for the 16-head MHA problem.

kernel(**inputs) takes FULL inputs (x [4,2048,1024], Wq/Wk/Wv [16,1024,64],
Wo [1024,1024], bo [1024]) and returns the FULL output [4,2048,1024] f32.

Sharding over 8 NeuronCores: core c handles batch b = c//2 and head group
g = c%2 (8 of 16 heads) — tensor parallel over heads with the output
projection's input dim sharded; the 2-way partial-sum reduce per batch and
the bias add happen host-side on the gathered results.
"""
import sys

for _p in ("/opt/trn_rl_repo",):
    if _p not in sys.path:
        sys.path.insert(0, _p)

import numpy as np
import concourse.bass as bass
import concourse.mybir as mybir
from concourse import bacc
from concourse.bass import ts, ds
from concourse.masks import make_identity
from concourse.tile import TileContext
from concourse.vector_clock import ScopedClock
from concourse import bass_utils

F32 = mybir.dt.float32
BF16 = mybir.dt.bfloat16
AF = mybir.ActivationFunctionType

NUM_HEADS = 16
EMB = 1024
HEAD = 64
SEQ = 2048
BATCH = 4
N_CORES = 8


class TC(TileContext):
    """TileContext whose final drain splits its sem waits across SP NOPs —
    the CTRL instruction encoding holds only one wait and this env's Tile
    puts the whole global clock on the tail drain."""

    def _drain_and_barrier(self, tick_clock, wait_clock):
        nc = self.nc
        dummy = nc.sync.nop(nofuse=True)
        wait_clock.add_sem_waits(dummy.ins, ScopedClock({None: tick_clock.global_clock}))
        si = dummy.ins.sync_info
        waits = list(si.on_wait) if si is not None else []
        if len(waits) > 1:
            si.on_wait = waits[:1]
            sem_by_name = {h.name: h for h in self.sems.allocated().values()}
            for w in waits[1:]:
                nop = nc.sync.nop(nofuse=True)
                nop._wait_ge(sem_by_name[w.ant_name], w.wait_value)
        nc.sync.drain()
        nc.all_engine_barrier()
        popped = nc._tile_sem_poison_stack.pop()
        assert popped is self._sem_poison
        nc.clear_and_free_semaphores(list(self.sems.allocated().values()))
        nc.all_engine_barrier()


def build_mha_nc(S=SEQ, E=EMB, D=HEAD, H=NUM_HEADS // 2):
    """Single-core SPMD program; H = heads per core (pair-packed).

    Fully transposed formulation:
      xT (PE-transposed once), qT/kT per pair (q pre-scaled by 1/sqrt(D)),
      scoresT = kT-slice.T @ qT-slice (K=64 row groups, both heads of a
      pair into one 2-bank PSUM tile),
      attnT = exp(scoresT) (scores bounded, no max subtraction; one ACT op
      and one causal affine_select per pair-tile),
      ctxT+denominator = [v|1].T @ attnT accumulated over k chunks,
      reciprocal of the denominator via DMA-reshape to [128, x] so all DVE
      lanes work, broadcast along d via a K=1 ones matmul,
      out = concatT-chunk.T @ WoT-chunk accumulated over head-dim chunks.

    Scheduling structure: q/k projections for pair p+1 are interleaved
    into pair p's attention groups to fill PE gaps; ce accumulators are
    tagged by q-chunk parity so a group's normalization tail overlaps the
    next group's matmuls.
    """
    P = 128
    EC = E // P
    NQ = 512
    J = S // NQ
    KK = S // P
    NP = H // 2
    HD = H * D
    HC = HD // P
    NE = min(512, E)
    JE = E // NE
    RQ = NQ // 64          # reshape width for the two-row reciprocal trick

    nc = bacc.Bacc("TRN2", target_bir_lowering=False, debug=False)
    x_b = nc.dram_tensor("x_b", [S, E], F32, kind="ExternalInput")
    Wq_s = nc.dram_tensor("Wq_s", [H, E, D], F32, kind="ExternalInput")
    Wk_s = nc.dram_tensor("Wk_s", [H, E, D], F32, kind="ExternalInput")
    Wv_s = nc.dram_tensor("Wv_s", [H, E, D], F32, kind="ExternalInput")
    Wo_s = nc.dram_tensor("Wo_s", [E, HD], F32, kind="ExternalInput")
    out_p = nc.dram_tensor("out_p", [S, E], F32, kind="ExternalOutput")

    scale = D ** -0.5

    with TC(nc) as tc:
        with (
            tc.tile_pool(name="const", bufs=1) as cpool,
            tc.tile_pool(name="persist", bufs=1) as pers,
            tc.tile_pool(name="stage", bufs=3) as stg,
            tc.tile_pool(name="wstage", bufs=2) as wstg_pool,
            tc.tile_pool(name="attn", bufs=6) as apool,
            tc.tile_pool(name="small", bufs=3) as spool,
            tc.tile_pool(name="psS", bufs=4, space="PSUM") as psS,
            tc.tile_pool(name="psC", bufs=1, space="PSUM") as psC,
        ):
            ident = cpool.tile([P, P], F32, tag="ident")
            make_identity(nc, ident[:])
            identb = cpool.tile([P, P], BF16, tag="identb")
            nc.vector.tensor_copy(identb[:], ident[:])
            ones_col = cpool.tile([1, D], mybir.dt.float16, tag="ones")
            nc.gpsimd.memset(ones_col[:], 1.0)

            xT = pers.tile([P, EC, S], BF16, tag="xT")
            qT = pers.tile([P, NP, S], BF16, tag="qT")
            kT = pers.tile([P, NP, S], BF16, tag="kT")
            v_pad = pers.tile([P, KK, H, D + 1], BF16, tag="vp")
            woT = pers.tile([P, HC, E], BF16, tag="woT")
            concatT = pers.tile([P, NP, S], BF16, tag="concT")
            wq_bf = pers.tile([P, NP, EC, 2 * D], BF16, tag="wq")
            wk_bf = pers.tile([P, NP, EC, 2 * D], BF16, tag="wk")
            wv_bf = pers.tile([P, EC, H, D], BF16, tag="wv")

            nc.gpsimd.memset(v_pad[:, :, :, D:D + 1], 1.0)

            # ---- Phase A: weight loads/casts first, then stream x with
            # transposes, v projection and pair-0 q/k fused in so PE has work
            # while DMA streams ----
            def emit_wqk(h):
                p2h, hh = h // 2, h % 2
                for w_dram, w_sb in ((Wq_s, wq_bf), (Wk_s, wk_bf)):
                    wstg = stg.tile([P, EC, D], F32, tag="wstg",
                                    name=f"wstg_{h}_{0 if w_dram is Wq_s else 1}")
                    nc.sync.dma_start(
                        wstg[:], w_dram[h].rearrange("(ec p) d -> p ec d", p=P)
                    )
                    nc.vector.tensor_copy(w_sb[:, p2h, :, ds(hh * D, D)], wstg[:])

            # hoist the first x tiles ahead of the weight streams
            xf_pre = {}
            for sc in range(2):
                xf = stg.tile([P, E], F32, tag="xstg", name=f"xf_pre{sc}")
                nc.sync.dma_start(xf[:], x_b[ts(sc, P), :])
                xf_pre[sc] = xf

            def emit_wv(h):
                wvs = wstg_pool.tile([P, EC, D], F32, tag="wvstg", name=f"wvstg_{h}")
                nc.sync.dma_start(
                    wvs[:], Wv_s[h].rearrange("(ec p) d -> p ec d", p=P)
                )
                nc.vector.tensor_copy(wv_bf[:, :, h, :], wvs[:])

            def emit_wqk(h):
                p2h, hh = h // 2, h % 2
                for w_dram, w_sb in ((Wq_s, wq_bf), (Wk_s, wk_bf)):
                    wstg = stg.tile([P, EC, D], F32, tag="wstg",
                                    name=f"wstg_{h}_{0 if w_dram is Wq_s else 1}")
                    nc.sync.dma_start(
                        wstg[:], w_dram[h].rearrange("(ec p) d -> p ec d", p=P)
                    )
                    nc.vector.tensor_copy(w_sb[:, p2h, :, ds(hh * D, D)], wstg[:])

            def emit_qk(p2, sc):
                # sc indexes NQ-wide chunks (two per call site index)
                for w_sb, dst, sc_mul in ((wq_bf, qT, scale), (wk_bf, kT, 1.0)):
                    acc = psS.tile(
                        [P, NQ], F32, tag="sc2",
                        name=f"qk_{p2}_{sc}_{0 if dst is qT else 1}",
                    )
                    for ec in range(EC):
                        nc.tensor.matmul(
                            acc[:],
                            w_sb[:, p2, ec, :],
                            xT[:, ec, ts(sc, NQ)],
                            start=(ec == 0), stop=(ec == EC - 1),
                        )
                    if sc_mul != 1.0:
                        nc.vector.tensor_scalar_mul(
                            dst[:, p2, ts(sc, NQ)], acc[:], sc_mul
                        )
                    else:
                        nc.vector.tensor_copy(dst[:, p2, ts(sc, NQ)], acc[:])

            def emit_v(sc):
                acc = psS.tile([P, HD], F32, tag="sc2", name=f"vacc_{sc}")
                for ec in range(EC):
                    nc.tensor.matmul(
                        acc[:],
                        xT[:, ec, ts(sc, P)],
                        wv_bf[:, ec, :, :].rearrange("p h d -> p (h d)"),
                        start=(ec == 0), stop=(ec == EC - 1),
                    )
                nc.vector.tensor_copy(
                    v_pad[:, sc, :, 0:D],
                    acc[:].rearrange("p (h d) -> p h d", d=D),
                )

            VDELAY = 4
            for sc in range(S // P):
                if sc in xf_pre:
                    xf = xf_pre.pop(sc)
                else:
                    xf = stg.tile([P, E], F32, tag="xstg", name=f"xf_{sc}")
                    nc.sync.dma_start(xf[:], x_b[ts(sc, P), :])
                # trickle weight DMAs between x-tile DMAs in queue order
                if sc < 4:
                    emit_wv(2 * sc)
                    emit_wv(2 * sc + 1)
                elif sc < 4 + H:
                    emit_wqk(sc - 4)
                xfb = stg.tile([P, E], BF16, tag="xfb", name=f"xfb_{sc}")
                nc.scalar.activation(xfb[:], xf[:], AF.Copy)
                for e4 in range(EC // 4):
                    pt = psS.tile([P, 4, P], BF16, tag="sc2", name=f"ptx_{sc}_{e4}")
                    for k in range(4):
                        nc.tensor.transpose(
                            pt[:, k, :], xfb[:, ts(4 * e4 + k, P)], identb[:]
                        )
                    nc.vector.tensor_copy(xT[:, 4 * e4:4 * e4 + 4, ts(sc, P)], pt[:])
                if sc >= VDELAY:
                    emit_v(sc - VDELAY)
                if sc % 4 == 3 and sc // 4 > 0:
                    emit_qk(0, sc // 4 - 1)
            for sc in range(S // P - VDELAY, S // P):
                emit_v(sc)
            emit_qk(0, J - 1)

            # Wo load + transpose (overlaps the start of attention)
            for ec in range(EC):
                wostg = stg.tile([P, HD], F32, tag="wostg")
                nc.sync.dma_start(wostg[:], Wo_s[ts(ec, P), :])
                for hc in range(HC):
                    pt = psS.tile([P, P], F32, tag="sc2", name=f"ptw_{ec}_{hc}")
                    nc.tensor.transpose(pt[:], wostg[:, ts(hc, P)], ident[:])
                    nc.vector.tensor_copy(woT[:, hc, ts(ec, P)], pt[:])

            # ---- per pair: attention, next pair's q/k interleaved, and each
            # group's normalization deferred behind the next group's body so
            # the PE never stalls on the reciprocal round trip ----
            def emit_out_chunks(j):
                for sc in range(4 * j, 4 * j + 4):
                    for n in range(JE):
                        acc = psS.tile([P, NE], F32, tag="sc2",
                                       name=f"oacc_{sc}_{n}")
                        for hc in range(HC):
                            nc.tensor.matmul(
                                acc[:],
                                concatT[:, hc, ts(sc, P)],
                                woT[:, hc, ts(n, NE)],
                                start=(hc == 0), stop=(hc == HC - 1),
                            )
                        ot = stg.tile([P, NE], F32, tag="ostg")
                        nc.vector.tensor_copy(ot[:], acc[:])
                        nc.sync.dma_start(out_p[ts(sc, P), ts(n, NE)], ot[:])

            def emit_normalize(p2, j, ce):
                dens = [
                    spool.tile([1, NQ], F32, tag=f"den{hh}",
                               name=f"den{hh}_{p2}_{j}")
                    for hh in range(2)
                ]
                for hh in range(2):
                    nc.vector.tensor_copy(dens[hh][:], ce[hh][ds(D, 1), :])
                den_rs = spool.tile([P, RQ], F32, tag="denrs")
                for hh in range(2):
                    nc.sync.dma_start(
                        den_rs[ds(hh * (P // 2), P // 2), :], dens[hh][:]
                    )
                rc = spool.tile([P, RQ], F32, tag="rc")
                nc.vector.reciprocal(rc[:], den_rs[:])
                rch = spool.tile([P, RQ], mybir.dt.float16, tag="rch")
                nc.vector.tensor_copy(rch[:], rc[:])
                recips = [
                    spool.tile([1, NQ], mybir.dt.float16, tag=f"recip{hh}",
                               name=f"recip{hh}_{p2}_{j}")
                    for hh in range(2)
                ]
                for hh in range(2):
                    nc.sync.dma_start(
                        recips[hh][:], rch[ds(hh * (P // 2), P // 2), :]
                    )
                for hh in range(2):
                    bc = psS.tile([D, NQ], F32, tag="sc2", name=f"bc_{p2}_{j}_{hh}")
                    nc.tensor.matmul(
                        bc[:], ones_col[:], recips[hh][:],
                        start=True, stop=True,
                    )
                    bc_sb = spool.tile([D, NQ], F32, tag="bcsb")
                    nc.vector.tensor_copy(bc_sb[:], bc[:])
                    nc.vector.tensor_mul(
                        concatT[ds(hh * D, D), p2, ts(j, NQ)],
                        ce[hh][0:D, :], bc_sb[:],
                    )

            pending = None
            for p2 in range(NP):
                h0, h1 = 2 * p2, 2 * p2 + 1
                for j in range(J):
                    n_kk = min(KK, 4 * j + 4)
                    ce = [
                        psC.tile([D + 1, NQ], F32, tag=f"ce{hh}{j % 2}",
                                 name=f"ce{hh}_{p2}_{j}")
                        for hh in range(2)
                    ]
                    for i in range(n_kk):
                        t = i - 4 * j  # >= 0 -> diagonal (partial) tile
                        # valid q range of this tile: q >= 128*t (tile-relative)
                        q0 = P * t if t > 0 else 0
                        nq = NQ - q0
                        for hh, hloc in enumerate((h0, h1)):
                            sps = psS.tile([P, NQ], F32, tag="sc2",
                                           name=f"s2_{p2}_{j}_{i}_{hh}")
                            nc.tensor.matmul(
                                sps[:, 0:nq],
                                kT[ds(hh * D, D), p2, ts(i, P)],
                                qT[ds(hh * D, D), p2, ds(j * NQ + q0, nq)],
                                start=True, stop=True,
                            )
                            at = apool.tile([P, NQ], BF16, tag="at")
                            nc.scalar.activation(at[:, 0:nq], sps[:, 0:nq], AF.Exp)
                            if t >= 0:
                                nc.gpsimd.affine_select(
                                    out=at[:, 0:nq], in_=at[:, 0:nq],
                                    compare_op=mybir.AluOpType.is_ge,
                                    fill=0.0, base=P * t - q0,
                                    pattern=[[1, nq]], channel_multiplier=-1,
                                )
                            nc.tensor.matmul(
                                ce[hh][0:D + 1, ds(q0, nq)],
                                v_pad[:, i, hloc, :],
                                at[:, 0:nq],
                                start=(i == 0), stop=(i == n_kk - 1),
                            )
                        if i == min(3, n_kk - 1) and pending is not None:
                            emit_normalize(*pending)
                            if pending[0] == NP - 1:
                                emit_out_chunks(pending[1])
                            pending = None
                    if pending is not None:
                        emit_normalize(*pending)
                        if pending[0] == NP - 1:
                            emit_out_chunks(pending[1])
                    pending = (p2, j, ce)
                    if p2 + 1 < NP:
                        emit_qk(p2 + 1, j)
            emit_normalize(*pending)
            emit_out_chunks(pending[1])

    nc.finalize()
    return nc


_NC_CACHE = {}


def _get_nc():
    key = "mha"
    if key not in _NC_CACHE:
        _NC_CACHE[key] = build_mha_nc()
    return _NC_CACHE[key]


def kernel(x, Wq, Wk, Wv, Wo, bo, _runner_kwargs=None):
    x = np.ascontiguousarray(np.asarray(x, dtype=np.float32))
    Wq = np.ascontiguousarray(np.asarray(Wq, dtype=np.float32))
    Wk = np.ascontiguousarray(np.asarray(Wk, dtype=np.float32))
    Wv = np.ascontiguousarray(np.asarray(Wv, dtype=np.float32))
    Wo = np.ascontiguousarray(np.asarray(Wo, dtype=np.float32))
    bo = np.asarray(bo, dtype=np.float32)

    HPC = NUM_HEADS // 2  # heads per core
    HDS = HPC * HEAD      # concat-dim slice per core

    nc = _get_nc()
    in_maps = []
    for c in range(N_CORES):
        b, g = c // 2, c % 2
        hs = slice(g * HPC, (g + 1) * HPC)
        in_maps.append({
            "x_b": x[b],
            "Wq_s": np.ascontiguousarray(Wq[hs]),
            "Wk_s": np.ascontiguousarray(Wk[hs]),
            "Wv_s": np.ascontiguousarray(Wv[hs]),
            "Wo_s": np.ascontiguousarray(Wo[:, g * HDS:(g + 1) * HDS]),
        })

    kw = dict(_runner_kwargs or {})
    res = bass_utils.run_bass_kernel_spmd(
        nc, in_maps, core_ids=list(range(N_CORES)), **kw
    )

    out = np.empty((BATCH, SEQ, EMB), dtype=np.float32)
    for b in range(BATCH):
        out[b] = res.results[2 * b]["out_p"] + res.results[2 * b + 1]["out_p"] + bo
    if kw.get("trace"):
        kernel.last_results = res
    return out


# revision 26
# speedup vs baseline: 1.0312x; 1.0312x over previous
"""Self-contained TRN2 Bass kernel for the 16-head MHA problem.

kernel(**inputs) takes FULL inputs (x [4,2048,1024], Wq/Wk/Wv [16,1024,64],
Wo [1024,1024], bo [1024]) and returns the FULL output [4,2048,1024] f32.

Sharding over 8 NeuronCores: core c handles batch b = c//2 and head group
g = c%2 (8 of 16 heads) — tensor parallel over heads with the output
projection's input dim sharded; the 2-way partial-sum reduce per batch and
the bias add happen host-side on the gathered results.
"""
import sys

for _p in ("/opt/trn_rl_repo",):
    if _p not in sys.path:
        sys.path.insert(0, _p)

import numpy as np
import concourse.bass as bass
import concourse.mybir as mybir
from concourse import bacc
from concourse.bass import ts, ds
from concourse.masks import make_identity
from concourse.tile import TileContext
from concourse.vector_clock import ScopedClock
from concourse import bass_utils

F32 = mybir.dt.float32
BF16 = mybir.dt.bfloat16
AF = mybir.ActivationFunctionType

NUM_HEADS = 16
EMB = 1024
HEAD = 64
SEQ = 2048
BATCH = 4
N_CORES = 8


class TC(TileContext):
    """TileContext whose final drain splits its sem waits across SP NOPs —
    the CTRL instruction encoding holds only one wait and this env's Tile
    puts the whole global clock on the tail drain."""

    def _drain_and_barrier(self, tick_clock, wait_clock):
        nc = self.nc
        dummy = nc.sync.nop(nofuse=True)
        wait_clock.add_sem_waits(dummy.ins, ScopedClock({None: tick_clock.global_clock}))
        si = dummy.ins.sync_info
        waits = list(si.on_wait) if si is not None else []
        if len(waits) > 1:
            si.on_wait = waits[:1]
            sem_by_name = {h.name: h for h in self.sems.allocated().values()}
            for w in waits[1:]:
                nop = nc.sync.nop(nofuse=True)
                nop._wait_ge(sem_by_name[w.ant_name], w.wait_value)
        nc.sync.drain()
        nc.all_engine_barrier()
        popped = nc._tile_sem_poison_stack.pop()
        assert popped is self._sem_poison
        nc.clear_and_free_semaphores(list(self.sems.allocated().values()))
        nc.all_engine_barrier()


def build_mha_nc(S=SEQ, E=EMB, D=HEAD, H=NUM_HEADS // 2):
    """Single-core SPMD program; H = heads per core (pair-packed).

    Fully transposed formulation:
      xT (PE-transposed once), qT/kT per pair (q pre-scaled by 1/sqrt(D)),
      scoresT = kT-slice.T @ qT-slice (K=64 row groups, both heads of a
      pair into one 2-bank PSUM tile),
      attnT = exp(scoresT) (scores bounded, no max subtraction; one ACT op
      and one causal affine_select per pair-tile),
      ctxT+denominator = [v|1].T @ attnT accumulated over k chunks,
      reciprocal of the denominator via DMA-reshape to [128, x] so all DVE
      lanes work, broadcast along d via a K=1 ones matmul,
      out = concatT-chunk.T @ WoT-chunk accumulated over head-dim chunks.

    Scheduling structure: q/k projections for pair p+1 are interleaved
    into pair p's attention groups to fill PE gaps; ce accumulators are
    tagged by q-chunk parity so a group's normalization tail overlaps the
    next group's matmuls.
    """
    P = 128
    EC = E // P
    NQ = 512
    J = S // NQ
    KK = S // P
    NP = H // 2
    HD = H * D
    HC = HD // P
    NE = min(512, E)
    JE = E // NE
    RQ = NQ // 64          # reshape width for the two-row reciprocal trick

    nc = bacc.Bacc("TRN2", target_bir_lowering=False, debug=False)
    x_b = nc.dram_tensor("x_b", [S, E], F32, kind="ExternalInput")
    Wq_s = nc.dram_tensor("Wq_s", [H, E, D], F32, kind="ExternalInput")
    Wk_s = nc.dram_tensor("Wk_s", [H, E, D], F32, kind="ExternalInput")
    Wv_s = nc.dram_tensor("Wv_s", [H, E, D], F32, kind="ExternalInput")
    Wo_s = nc.dram_tensor("Wo_s", [E, HD], F32, kind="ExternalInput")
    out_p = nc.dram_tensor("out_p", [S, E], F32, kind="ExternalOutput")

    scale = D ** -0.5

    with TC(nc) as tc:
        with (
            tc.tile_pool(name="const", bufs=1) as cpool,
            tc.tile_pool(name="persist", bufs=1) as pers,
            tc.tile_pool(name="stage", bufs=3) as stg,
            tc.tile_pool(name="wstage", bufs=2) as wstg_pool,
            tc.tile_pool(name="attn", bufs=6) as apool,
            tc.tile_pool(name="small", bufs=3) as spool,
            tc.tile_pool(name="psS", bufs=4, space="PSUM") as psS,
            tc.tile_pool(name="psC", bufs=1, space="PSUM") as psC,
        ):
            ident = cpool.tile([P, P], F32, tag="ident")
            make_identity(nc, ident[:])
            identb = cpool.tile([P, P], BF16, tag="identb")
            nc.vector.tensor_copy(identb[:], ident[:])
            ones_col = cpool.tile([1, D], mybir.dt.float16, tag="ones")
            nc.gpsimd.memset(ones_col[:], 1.0)

            xT = pers.tile([P, EC, S], BF16, tag="xT")
            qT = pers.tile([P, NP, S], BF16, tag="qT")
            kT = pers.tile([P, NP, S], BF16, tag="kT")
            v_pad = pers.tile([P, KK, H, D + 1], BF16, tag="vp")
            woT = pers.tile([P, HC, E], BF16, tag="woT")
            concatT = pers.tile([P, NP, S], BF16, tag="concT")
            wq_bf = pers.tile([P, NP, EC, 2 * D], BF16, tag="wq")
            wk_bf = pers.tile([P, NP, EC, 2 * D], BF16, tag="wk")
            wv_bf = pers.tile([P, EC, H, D], BF16, tag="wv")

            nc.gpsimd.memset(v_pad[:, :, :, D:D + 1], 1.0)

            # ---- Phase A: weight loads/casts first, then stream x with
            # transposes, v projection and pair-0 q/k fused in so PE has work
            # while DMA streams ----
            def emit_wqk(h):
                p2h, hh = h // 2, h % 2
                for w_dram, w_sb in ((Wq_s, wq_bf), (Wk_s, wk_bf)):
                    wstg = stg.tile([P, EC, D], F32, tag="wstg",
                                    name=f"wstg_{h}_{0 if w_dram is Wq_s else 1}")
                    nc.sync.dma_start(
                        wstg[:], w_dram[h].rearrange("(ec p) d -> p ec d", p=P)
                    )
                    nc.vector.tensor_copy(w_sb[:, p2h, :, ds(hh * D, D)], wstg[:])

            # hoist the first x tiles ahead of the weight streams
            xf_pre = {}
            for sc in range(2):
                xf = stg.tile([P, E], F32, tag="xstg", name=f"xf_pre{sc}")
                nc.sync.dma_start(xf[:], x_b[ts(sc, P), :])
                xf_pre[sc] = xf

            def emit_wv(h):
                wvs = wstg_pool.tile([P, EC, D], F32, tag="wvstg", name=f"wvstg_{h}")
                nc.sync.dma_start(
                    wvs[:], Wv_s[h].rearrange("(ec p) d -> p ec d", p=P)
                )
                nc.vector.tensor_copy(wv_bf[:, :, h, :], wvs[:])

            def emit_wqk(h):
                p2h, hh = h // 2, h % 2
                for w_dram, w_sb in ((Wq_s, wq_bf), (Wk_s, wk_bf)):
                    wstg = stg.tile([P, EC, D], F32, tag="wstg",
                                    name=f"wstg_{h}_{0 if w_dram is Wq_s else 1}")
                    nc.sync.dma_start(
                        wstg[:], w_dram[h].rearrange("(ec p) d -> p ec d", p=P)
                    )
                    nc.vector.tensor_copy(w_sb[:, p2h, :, ds(hh * D, D)], wstg[:])

            def emit_qk(p2, sc):
                # sc indexes NQ-wide chunks (two per call site index)
                for w_sb, dst, sc_mul in ((wq_bf, qT, scale), (wk_bf, kT, 1.0)):
                    acc = psS.tile(
                        [P, NQ], F32, tag="sc2",
                        name=f"qk_{p2}_{sc}_{0 if dst is qT else 1}",
                    )
                    for ec in range(EC):
                        nc.tensor.matmul(
                            acc[:],
                            w_sb[:, p2, ec, :],
                            xT[:, ec, ts(sc, NQ)],
                            start=(ec == 0), stop=(ec == EC - 1),
                        )
                    if sc_mul != 1.0:
                        nc.vector.tensor_scalar_mul(
                            dst[:, p2, ts(sc, NQ)], acc[:], sc_mul
                        )
                    else:
                        nc.vector.tensor_copy(dst[:, p2, ts(sc, NQ)], acc[:])

            def emit_v(sc):
                acc = psS.tile([P, HD], F32, tag="sc2", name=f"vacc_{sc}")
                for ec in range(EC):
                    nc.tensor.matmul(
                        acc[:],
                        xT[:, ec, ts(sc, P)],
                        wv_bf[:, ec, :, :].rearrange("p h d -> p (h d)"),
                        start=(ec == 0), stop=(ec == EC - 1),
                    )
                nc.vector.tensor_copy(
                    v_pad[:, sc, :, 0:D],
                    acc[:].rearrange("p (h d) -> p h d", d=D),
                )

            def emit_out_chunks(j):
                for sc in range(4 * j, 4 * j + 4):
                    for n in range(JE):
                        acc = psS.tile([P, NE], F32, tag="sc2",
                                       name=f"oacc_{sc}_{n}")
                        for hc in range(HC):
                            nc.tensor.matmul(
                                acc[:],
                                concatT[:, hc, ts(sc, P)],
                                woT[:, hc, ts(n, NE)],
                                start=(hc == 0), stop=(hc == HC - 1),
                            )
                        ot = stg.tile([P, NE], F32, tag="ostg")
                        nc.vector.tensor_copy(ot[:], acc[:])
                        nc.sync.dma_start(out_p[ts(sc, P), ts(n, NE)], ot[:])

            def emit_normalize(p2, j, ce):
                dens = [
                    spool.tile([1, NQ], F32, tag=f"den{hh}",
                               name=f"den{hh}_{p2}_{j}")
                    for hh in range(2)
                ]
                for hh in range(2):
                    nc.vector.tensor_copy(dens[hh][:], ce[hh][ds(D, 1), :])
                den_rs = spool.tile([P, RQ], F32, tag="denrs")
                for hh in range(2):
                    nc.sync.dma_start(
                        den_rs[ds(hh * (P // 2), P // 2), :], dens[hh][:]
                    )
                rc = spool.tile([P, RQ], F32, tag="rc")
                nc.vector.reciprocal(rc[:], den_rs[:])
                rch = spool.tile([P, RQ], mybir.dt.float16, tag="rch")
                nc.vector.tensor_copy(rch[:], rc[:])
                recips = [
                    spool.tile([1, NQ], mybir.dt.float16, tag=f"recip{hh}",
                               name=f"recip{hh}_{p2}_{j}")
                    for hh in range(2)
                ]
                for hh in range(2):
                    nc.sync.dma_start(
                        recips[hh][:], rch[ds(hh * (P // 2), P // 2), :]
                    )
                for hh in range(2):
                    bc = psS.tile([D, NQ], F32, tag="sc2", name=f"bc_{p2}_{j}_{hh}")
                    nc.tensor.matmul(
                        bc[:], ones_col[:], recips[hh][:],
                        start=True, stop=True,
                    )
                    bc_sb = spool.tile([D, NQ], F32, tag="bcsb")
                    nc.vector.tensor_copy(bc_sb[:], bc[:])
                    nc.vector.tensor_mul(
                        concatT[ds(hh * D, D), p2, ts(j, NQ)],
                        ce[hh][0:D, :], bc_sb[:],
                    )

            pending = [None]

            def flush_pending():
                if pending[0] is not None:
                    p2x, jx, cex = pending[0]
                    emit_normalize(p2x, jx, cex)
                    if p2x == NP - 1:
                        emit_out_chunks(jx)
                    pending[0] = None

            def emit_group(p2, j):
                h0, h1 = 2 * p2, 2 * p2 + 1
                n_kk = min(KK, 4 * j + 4)
                ce = [
                    psC.tile([D + 1, NQ], F32, tag=f"ce{hh}{j % 2}",
                             name=f"ce{hh}_{p2}_{j}")
                    for hh in range(2)
                ]
                for i in range(n_kk):
                    t = i - 4 * j  # >= 0 -> diagonal (partial) tile
                    q0 = P * t if t > 0 else 0
                    nq = NQ - q0
                    for hh, hloc in enumerate((h0, h1)):
                        sps = psS.tile([P, NQ], F32, tag="sc2",
                                       name=f"s2_{p2}_{j}_{i}_{hh}")
                        nc.tensor.matmul(
                            sps[:, 0:nq],
                            kT[ds(hh * D, D), p2, ts(i, P)],
                            qT[ds(hh * D, D), p2, ds(j * NQ + q0, nq)],
                            start=True, stop=True,
                        )
                        at = apool.tile([P, NQ], BF16, tag="at")
                        nc.scalar.activation(at[:, 0:nq], sps[:, 0:nq], AF.Exp)
                        if t >= 0:
                            nc.gpsimd.affine_select(
                                out=at[:, 0:nq], in_=at[:, 0:nq],
                                compare_op=mybir.AluOpType.is_ge,
                                fill=0.0, base=P * t - q0,
                                pattern=[[1, nq]], channel_multiplier=-1,
                            )
                        nc.tensor.matmul(
                            ce[hh][0:D + 1, ds(q0, nq)],
                            v_pad[:, i, hloc, :],
                            at[:, 0:nq],
                            start=(i == 0), stop=(i == n_kk - 1),
                        )
                    if i == min(3, n_kk - 1):
                        flush_pending()
                flush_pending()
                pending[0] = (p2, j, ce)

            VDELAY = 4
            for sc in range(S // P):
                if sc in xf_pre:
                    xf = xf_pre.pop(sc)
                else:
                    xf = stg.tile([P, E], F32, tag="xstg", name=f"xf_{sc}")
                    nc.sync.dma_start(xf[:], x_b[ts(sc, P), :])
                # trickle weight DMAs between x-tile DMAs in queue order
                if sc < 4:
                    emit_wv(2 * sc)
                    emit_wv(2 * sc + 1)
                elif sc < 4 + H:
                    emit_wqk(sc - 4)
                xfb = stg.tile([P, E], BF16, tag="xfb", name=f"xfb_{sc}")
                nc.scalar.activation(xfb[:], xf[:], AF.Copy)
                for e4 in range(EC // 4):
                    pt = psS.tile([P, 4, P], BF16, tag="sc2", name=f"ptx_{sc}_{e4}")
                    for k in range(4):
                        nc.tensor.transpose(
                            pt[:, k, :], xfb[:, ts(4 * e4 + k, P)], identb[:]
                        )
                    nc.vector.tensor_copy(xT[:, 4 * e4:4 * e4 + 4, ts(sc, P)], pt[:])
                if sc >= VDELAY:
                    emit_v(sc - VDELAY)
                if sc % 4 == 3 and sc // 4 > 0:
                    emit_qk(0, sc // 4 - 1)
                if sc == 9:
                    emit_group(0, 0)
                if sc == 13:
                    emit_group(0, 1)
            for sc in range(S // P - VDELAY, S // P):
                emit_v(sc)
            emit_qk(0, J - 1)

            # Wo load + transpose (overlaps the start of attention)
            for ec in range(EC):
                wostg = stg.tile([P, HD], F32, tag="wostg")
                nc.sync.dma_start(wostg[:], Wo_s[ts(ec, P), :])
                for hc in range(HC):
                    pt = psS.tile([P, P], F32, tag="sc2", name=f"ptw_{ec}_{hc}")
                    nc.tensor.transpose(pt[:], wostg[:, ts(hc, P)], ident[:])
                    nc.vector.tensor_copy(woT[:, hc, ts(ec, P)], pt[:])

            # ---- remaining attention groups (pair 0 groups 0-1 were emitted
            # inside the x loop); next pair's q/k interleaved ----
            for p2 in range(NP):
                for j in range(J):
                    if p2 == 0 and j < 2:
                        emit_qk(1, j)
                        continue
                    emit_group(p2, j)
                    if p2 + 1 < NP:
                        emit_qk(p2 + 1, j)
            flush_pending()

    nc.finalize()
    return nc


_NC_CACHE = {}


def _get_nc():
    key = "mha"
    if key not in _NC_CACHE:
        _NC_CACHE[key] = build_mha_nc()
    return _NC_CACHE[key]


def kernel(x, Wq, Wk, Wv, Wo, bo, _runner_kwargs=None):
    x = np.ascontiguousarray(np.asarray(x, dtype=np.float32))
    Wq = np.ascontiguousarray(np.asarray(Wq, dtype=np.float32))
    Wk = np.ascontiguousarray(np.asarray(Wk, dtype=np.float32))
    Wv = np.ascontiguousarray(np.asarray(Wv, dtype=np.float32))
    Wo = np.ascontiguousarray(np.asarray(Wo, dtype=np.float32))
    bo = np.asarray(bo, dtype=np.float32)

    HPC = NUM_HEADS // 2  # heads per core
    HDS = HPC * HEAD      # concat-dim slice per core

    nc = _get_nc()
    in_maps = []
    for c in range(N_CORES):
        b, g = c // 2, c % 2
        hs = slice(g * HPC, (g + 1) * HPC)
        in_maps.append({
            "x_b": x[b],
            "Wq_s": np.ascontiguousarray(Wq[hs]),
            "Wk_s": np.ascontiguousarray(Wk[hs]),
            "Wv_s": np.ascontiguousarray(Wv[hs]),
            "Wo_s": np.ascontiguousarray(Wo[:, g * HDS:(g + 1) * HDS]),
        })

    kw = dict(_runner_kwargs or {})
    res = bass_utils.run_bass_kernel_spmd(
        nc, in_maps, core_ids=list(range(N_CORES)), **kw
    )

    out = np.empty((BATCH, SEQ, EMB), dtype=np.float32)
    for b in range(BATCH):
        out[b] = res.results[2 * b]["out_p"] + res.results[2 * b + 1]["out_p"] + bo
    if kw.get("trace"):
        kernel.last_results = res
    return out
